# revision 69
# baseline (speedup 1.0000x reference)
"""Euler-characteristic-curve kernel for Trainium2 (Bass/Tile).

Algorithm (cumulative-step histogram)
-------------------------------------
Per (batch, channel) group, reference computes
    cover(t_k) = #{n : birth_n < t_k <= death_n},  t_k = k/255, k=0..255
and the output is cover_pd0 - cover_pd1.

Identity: [b < t][d >= t] = [b < t] - [max(b,d) < t], so with the 256-bin
index q(v) = floor(255 v) (fp32 magic-add round, exact on sim and hw; the
1-pass int16 convert is NOT usable -- CoreSim truncates where silicon
rounds), each of the four streams (b0, m0, b1, m1) needs its cumulative
histogram C(k) = #{q < k}, folded with signs +,-,-,+.  max(b,d) runs as an
int16 pair-max on q (floor is monotone).

Step functions of the nibbles h = q >> 4, l = q & 15:
    A[:, j] = [h <= j],  B[:, j] = [l <= j]   (j = 0..15, {0,1} bf16)
make the point-contraction matmul A^T B accumulate the 2-D cumulative
joint histogram  cum2[K, L] = #{h <= K and l <= L}  directly.  Signs fold
into two PSUM chains (P+ = b0,m1; P- = m0,b1); net = P+ - P- and the
curve is the 3-term stencil
    C(16K+L) = net[K-1,15] + net[K,L-1] - net[K-1,L-1]
via one superdiagonal-shift matmul (m1 = eye(k=1)^T net) plus a single
fused scalar_tensor_tensor per group -- no prefix scans, no triangular
matmul, and a 1-subtract extraction instead of 4-slot sign folding.

Step emission is one instruction per bin j covering both nibbles and both
diagrams (c-packed): DVE tensor_scalar is_le runs in the 4x perf mode;
ACT emits steps in a SINGLE pass as Sigmoid(-256 (x - j - 0.5)), exact
{0,1} in bf16 (vs 2-pass Abs+Relu one-hots -- ACT takes 3 bins in half 0
and 4 in half 1); GPSIMD takes 2-3 bins.  The engine split, convert
placement, granule sizes and hook position were tuned against the
InstructionCostModel timeline; p-state warm-up matmuls measured as
unnecessary under this cost model (NWARM=0).

Sharding: data-parallel over batch, 4 batches per core x 8 cores.
"""

import os
import sys

for _p in ("/opt/trn_rl_repo", os.path.expanduser("~/.axon_site/_ro/trn_rl_repo")):
    if os.path.isdir(_p) and _p not in sys.path:
        sys.path.insert(0, _p)

import numpy as np

import concourse.bass as bass
import concourse.bacc as bacc
import concourse.mybir as mybir
from concourse.tile import TileContext
from concourse.bass_utils import run_bass_kernel_spmd

NCORES = 8
B, C, N = 32, 3, 8192
TT = 256                      # thresholds
NG = (B // NCORES) * C        # 12 groups (b,c pairs) per diagram per core
NI = N // 128                 # 64 point-slices of 128 per group
GSET = 4                      # groups packed per set
NSET = NG // GSET             # 3 sets per diagram
SETS = tuple((GSET * s, GSET) for s in range(NSET))
HALVES = ((0, 2), (2, 4))     # granules within a set

F32 = mybir.dt.float32
BF16 = mybir.dt.bfloat16
I16 = mybir.dt.int16
OP = mybir.AluOpType
AF = mybir.ActivationFunctionType

ACT_BINS = (3, 8, 13)
POOL_BINS = (14, 15)
# q-convert engine per diagram d (sets >= 1): "act"/"dve"/"pool"
CONV_ENG = ("dve", "act")
NWARM = 0
NWARM_MID = {}
FIN_ENG = "pool"          # engine for the fin assembly in non-tail finishes
PM_BUFS = 4               # m1 PSUM pool buffers (warm bank freed)
Q_BUFS = 4                # post SBUF pool buffers
NIB_HOOK = False          # nibbles of set si+1 at the j==10 hook vs set top
TAIL_MODE = "stage"       # "stage": 2-stage tail; "group"; "half"
FINISH_MODE = "set"       # granularity of deferred finishes: "half"/"set"
ACT_BINS_H1 = (3, 6, 9, 13)  # per-half override for ACT bins in half 1
POOL_BINS_H1 = (12, 14, 15)  # per-half override for POOL bins in half 1
H0_HALVES = ((0, 2), (2, 4))  # set-0 granules
HOOK_J = 6                    # pass index where the next-set hook fires
LAST_HALVES = None            # optional override for the last set's granules
MERGE_PREP = False            # single both-diagram magic/max for sets > 0 (worse)
MERGE_NIB = False             # single both-diagram nibble ops for sets > 0 (worse)
PP_BUFS = 4               # PSUM chain-accumulator buffers
MINUS_ALL = False         # minus-first chain order for every half
SPLIT_HOOK = False        # hook ACT phase at j=6, DVE phase between halves
FIN_AFTER = True          # emit deferred finishes after this half's chains


def build_nc(**over):
    g = globals()
    saved = {k: g[k] for k in over}
    g.update(over)
    try:
        return _build_nc()
    finally:
        g.update(saved)


def _build_nc():
    nc = bacc.Bacc("TRN2", target_bir_lowering=False, debug=False)
    pds = [
        nc.dram_tensor(f"pd{d}", [NG, N, 2], F32, kind="ExternalInput")
        for d in range(2)
    ]
    tri_d = nc.dram_tensor("tri", [16, 16], F32, kind="ExternalInput")
    out_d = nc.dram_tensor("out", [NG, TT], F32, kind="ExternalOutput")

    with TileContext(nc) as tc:
        with (
            tc.tile_pool(name="consts", bufs=1) as cpool,
            tc.tile_pool(name="src", bufs=2) as spool,
            tc.tile_pool(name="tmp", bufs=2) as tpool,
            tc.tile_pool(name="oh", bufs=2) as ohpool,
            tc.tile_pool(name="psum", bufs=PP_BUFS, space="PSUM") as ppool,
            tc.tile_pool(name="psm", bufs=PM_BUFS, space="PSUM") as pmpool,
            tc.tile_pool(name="post", bufs=Q_BUFS) as qpool,
        ):
            tri = cpool.tile([16, 16], F32)
            warm = cpool.tile([128, 1], F32)
            bias_sig = {}
            for j in set(ACT_BINS) | set(ACT_BINS_H1 or ()):
                bias_sig[j] = cpool.tile([128, 1], F32, name=f"bsig{j}")
                nc.vector.memset(bias_sig[j][:, :], 256.0 * (j + 0.5))

            state = {}   # si -> src / qt
            nib = {}     # si -> (hl, AB, ps_p, ps_m)

            # persistent step tiles: the j=15 plane of both nibbles is the
            # constant 1.0 step [x <= 15]; memset once (Pool idles through
            # the fill) so every set skips its j=15 pass
            ab_tiles = [
                cpool.tile([128, GSET, 16, 2, 2, 128], BF16, name=f"ABt{i}")
                for i in range(2)
            ]
            for t in ab_tiles:
                nc.gpsimd.memset(t[:, :, 15, :, :, :], 1.0)

            def emit_dma(si, dd=(0, 1)):
                g0, gs = SETS[si]
                src = state.get(si)
                if src is None:
                    src = spool.tile([128, 2, GSET, 128], F32, tag="src")
                    state[si] = src
                for d in dd:
                    nc.sync.dma_start(
                        src[:, d, 0:gs, :],
                        pds[d]
                        .ap()[g0 : g0 + gs, :, :]
                        .rearrange("g (p x) two -> p g (x two)", p=128),
                    )

            def emit_prep(si, dd=(0, 1), phase="all"):
                # q = floor(255 v) via the fp32 magic-add (exact on sim AND
                # hw -- the direct int16 convert truncates in CoreSim but
                # rounds on silicon, so a 1-pass convert cannot match both).
                # Then int16 pair-max folds death -> max(birth, death)
                # (floor is monotone).
                g0, gs = SETS[si]
                src = state[si]
                if si in nib:
                    tmb, qt = nib[si][0], nib[si][1]
                else:
                    tmb = tpool.tile([128, 2, GSET, 128], F32, tag="tmb")
                    qt = tpool.tile([128, 2, GSET, 128], I16, tag="qt")
                    nib[si] = (tmb, qt)
                if si > 0 and dd == (0, 1) and MERGE_PREP:
                    # prefetched set: latency is hidden, so spend fewer
                    # quanta -- one magic / conv-split / one max
                    nc.scalar.activation(
                        tmb[:, :, 0:gs, :], src[:, :, 0:gs, :],
                        AF.Copy, bias=8388607.5, scale=255.0,
                    )
                    for d in (0, 1):
                        eng = CONV_ENG[d]
                        if eng == "act":
                            nc.scalar.activation(
                                qt[:, d, 0:gs, :], tmb[:, d, 0:gs, :],
                                AF.Copy, bias=-8388608.0,
                            )
                        else:
                            e = nc.vector if eng == "dve" else nc.gpsimd
                            e.tensor_scalar(
                                qt[:, d, 0:gs, :], tmb[:, d, 0:gs, :],
                                8388608.0, None, OP.subtract,
                            )
                    pr = qt[:, :, 0:gs, :].rearrange(
                        "p d g (i two) -> p (d g i) two", two=2
                    )
                    nc.vector.tensor_tensor(
                        pr[:, :, 1:2], pr[:, :, 0:1], pr[:, :, 1:2], OP.max
                    )
                    return
                for d in dd:
                    if phase in ("all", "act"):
                        nc.scalar.activation(
                            tmb[:, d, 0:gs, :], src[:, d, 0:gs, :],
                            AF.Copy, bias=8388607.5, scale=255.0,
                        )
                        if CONV_ENG[d] == "act":
                            nc.scalar.activation(
                                qt[:, d, 0:gs, :], tmb[:, d, 0:gs, :],
                                AF.Copy, bias=-8388608.0,
                            )
                    if phase in ("all", "dve"):
                        if CONV_ENG[d] != "act":
                            e = nc.vector if CONV_ENG[d] == "dve" \
                                else nc.gpsimd
                            e.tensor_scalar(
                                qt[:, d, 0:gs, :], tmb[:, d, 0:gs, :],
                                8388608.0, None, OP.subtract,
                            )
                        pr = qt[:, d, 0:gs, :].rearrange(
                            "p g (i two) -> p (g i) two", two=2
                        )
                        nc.vector.tensor_tensor(
                            pr[:, :, 1:2], pr[:, :, 0:1], pr[:, :, 1:2],
                            OP.max
                        )

            def emit_nibbles(si, dd=(0, 1)):
                # hl[p, g, c, d, x]: c=0 hi nibble, c=1 lo nibble
                qt = nib[si][1]
                if len(nib[si]) == 2:
                    hl = tpool.tile([128, GSET, 2, 2, 128], I16, tag="hl")
                    AB = ab_tiles[si % 2]
                    ps_p = ppool.tile([16, GSET, 16], F32, tag="ps")
                    ps_m = ppool.tile([16, GSET, 16], F32, tag="ps")
                    nib[si] = (nib[si][0], qt, hl, AB, ps_p, ps_m)
                hl = nib[si][2]
                if si > 0 and dd == (0, 1) and MERGE_NIB:
                    # read qt[p, d, g, x] in (g, d, x) order to line up with
                    # hl[p, g, c, d, x]: strides permute, last dim packed
                    qa = qt[:, :, :, :]
                    qt_gd = bass.AP(
                        qa.tensor, qa.offset,
                        [qa.ap[0], [128, GSET], [GSET * 128, 2], [1, 128]],
                    )
                    nc.vector.tensor_scalar(
                        hl[:, :, 0, :, :], qt_gd, 4, None,
                        OP.logical_shift_right,
                    )
                    nc.vector.tensor_scalar(
                        hl[:, :, 1, :, :], qt_gd, 15, None, OP.bitwise_and
                    )
                    return
                for d in dd:
                    qd = qt[:, d, :, :]
                    qt_g = bass.AP(
                        qd.tensor, qd.offset,
                        [qd.ap[0], [128, GSET], [1, 128]],
                    )
                    nc.vector.tensor_scalar(
                        hl[:, :, 0, d, :], qt_g, 4, None,
                        OP.logical_shift_right,
                    )
                    nc.vector.tensor_scalar(
                        hl[:, :, 1, d, :], qt_g, 15, None, OP.bitwise_and
                    )

            def emit_half_bins(si, ga, gb, hook=None, hi=0):
                hl, AB = nib[si][2], nib[si][3]
                act_bins = ACT_BINS if (hi == 0 or ACT_BINS_H1 is None) \
                    else ACT_BINS_H1
                pool_bins = POOL_BINS if (hi == 0 or POOL_BINS_H1 is None) \
                    else POOL_BINS_H1
                for j in range(15):
                    if j == HOOK_J and hook is not None:
                        hook()
                    if j in act_bins:
                        nc.scalar.activation(
                            AB[:, ga:gb, j, :, :, :], hl[:, ga:gb, :, :, :],
                            AF.Sigmoid, bias=bias_sig[j][:, 0:1], scale=-256.0,
                        )
                    else:
                        eng = nc.gpsimd if j in pool_bins else nc.vector
                        eng.tensor_scalar(
                            AB[:, ga:gb, j, :, :, :],
                            hl[:, ga:gb, :, :, :],
                            j, None, OP.is_le,
                        )

            def emit_matmuls(si, ga, gb, minus_first=False):
                AB, ps_p, ps_m = nib[si][3], nib[si][4], nib[si][5]
                order = ((1, ps_m), (0, ps_p)) if minus_first else \
                    ((0, ps_p), (1, ps_m))
                for sgn, ps in order:
                    for g in range(ga, gb):
                        nlinks = 0
                        for d in range(2):
                            v = sgn ^ d  # +: (d0,b),(d1,m); -: (d0,m),(d1,b)
                            for i in range(NI):
                                x = 2 * i + v
                                nc.tensor.matmul(
                                    ps[:, g, :],
                                    AB[:, g, :, 0, d, x],
                                    AB[:, g, :, 1, d, x],
                                    start=(nlinks == 0),
                                    stop=(nlinks == 2 * NI - 1),
                                )
                                nlinks += 1

            pending = []

            def _finish(item, tail=False, fin=None, dma=True):
                # net = P+ - P- for groups [ga, gb), then the stencil
                #   fin[K,L] = net[K,L-1] - net[K-1,L-1] + net[K-1,15]
                # with net[K-1,*] = m1 = eye(k=1)^T net from the PE.
                si, g0, ga, gb = item
                ps_p, ps_m = nib[si][4], nib[si][5]
                sm = qpool.tile([16, GSET, 16], F32, tag="sm")
                nc.scalar.copy(sm[:, ga:gb, :], ps_m[:, ga:gb, :])
                # net17: col 0 is zero so the shifted m1 = tri^T net17 has
                # zero boundaries and ONE 16-wide STT per group writes the
                # whole output row (no per-group boundary copy)
                net = qpool.tile([16, GSET, 17], F32, tag="net")
                nc.vector.memset(net[:, ga:gb, 0:1], 0.0)
                nc.vector.tensor_tensor(
                    net[:, ga:gb, 1:17], ps_p[:, ga:gb, :], sm[:, ga:gb, :],
                    OP.subtract,
                )
                m1 = pmpool.tile([16, GSET, 17], F32, tag="m1")
                nc.tensor.matmul(
                    m1[:, ga:gb, :], tri[:, :], net[:, ga:gb, :],
                    start=True, stop=True,
                )
                if fin is None:
                    fin = qpool.tile([16, GSET, 16], F32, tag="fin")
                for gl in range(ga, gb):
                    # fin[:,0:16] = (net17[:,0:16] + m1[:,16]) - m1[:,0:16]
                    nc.vector.scalar_tensor_tensor(
                        fin[:, gl, 0:16], net[:, gl, 0:16],
                        m1[:, gl, 16:17], m1[:, gl, 0:16],
                        OP.add, OP.subtract,
                    )
                if dma:
                    nc.sync.dma_start(
                        out_d.ap()[g0 + ga : g0 + gb, :].rearrange(
                            "g (K L) -> K g L", K=16
                        ),
                        fin[:, ga:gb, :],
                    )
                return fin

            # ---- fill: both set-0 diagram DMAs lead (d1's arrival gates
            # the first half's c-packed passes)
            emit_dma(0)
            nc.sync.dma_start(tri[:, :], tri_d.ap())
            # preload the ACT Copy+Sigmoid table behind the first DMA
            nc.vector.memset(warm[:, :], 0.0)
            nc.scalar.mul(warm[:, :], warm[:, :], 2.0)
            nc.scalar.activation(
                warm[:, :], warm[:, :], AF.Sigmoid,
                bias=bias_sig[ACT_BINS[0]][:, 0:1],
            )
            if NWARM:
                wsrc = cpool.tile([128, 16], BF16)
                nc.vector.memset(wsrc[:, :], 0.0)
                wps = pmpool.tile([16, 16], F32, tag="warm_ps")
                for _ in range(NWARM):
                    nc.tensor.matmul(
                        wps[:, :], wsrc[:, :], wsrc[:, :], start=True,
                        stop=True,
                    )
            emit_dma(1)
            emit_prep(0)
            emit_nibbles(0)

            for si, (g0, gs) in enumerate(SETS):
                def hook(si=si):
                    if si + 2 < NSET:
                        emit_dma(si + 2)
                    emit_prep(si + 1, phase="act" if SPLIT_HOOK else "all")
                    if NIB_HOOK:
                        emit_nibbles(si + 1)

                if si > 0 and not NIB_HOOK:
                    emit_nibbles(si)
                halves = H0_HALVES if si == 0 else HALVES
                if si == NSET - 1 and LAST_HALVES is not None:
                    halves = LAST_HALVES
                for hi, (ga, gb) in enumerate(halves):
                    emit_half_bins(
                        si, ga, gb, hi=hi,
                        hook=hook if (si + 1 < NSET and hi == 0) else None,
                    )
                    if not FIN_AFTER:
                        while pending:
                            _finish(pending.pop(0))
                    last_half = si == NSET - 1 and hi == len(halves) - 1
                    ta = ga
                    if (SPLIT_HOOK and hi == 1 and si + 1 < NSET
                            and not (si == 0 and len(H0_HALVES) > 2)):
                        # DVE-phase prep of the next set lands AFTER this
                        # set's h1 passes in the DVE queue
                        emit_prep(si + 1, phase="dve")
                    if last_half and FIN_AFTER:
                        while pending:
                            _finish(pending.pop(0))
                    if last_half and TAIL_MODE == "group":
                        # per-group finishes overlap the last matmuls; a
                        # single batched DMA avoids serialized HWDGE setups
                        fin = None
                        for g in range(ta, gb):
                            if g >= ga:
                                emit_matmuls(si, g, g + 1)
                            fin = _finish(
                                (si, g0, g, g + 1), tail=(g + 1 == gb),
                                fin=fin, dma=False,
                            )
                        nc.sync.dma_start(
                            out_d.ap()[g0 + ta : g0 + gb, :].rearrange(
                                "g (K L) -> K g L", K=16
                            ),
                            fin[:, ta:gb, :],
                        )
                    elif last_half and TAIL_MODE == "stage":
                        # stage A (sm copy + net subtract, no PE) per group
                        # runs during the next group's chains; the PE-side
                        # m1 + assembly + DMA batch at the very end
                        ps_p, ps_m = nib[si][4], nib[si][5]
                        sm = qpool.tile([16, GSET, 16], F32, tag="sm")
                        net = qpool.tile([16, GSET, 17], F32, tag="net")
                        nc.vector.memset(net[:, ga:gb, 0:1], 0.0)
                        for g in range(ga, gb):
                            emit_matmuls(si, g, g + 1, minus_first=True)
                            nc.scalar.copy(sm[:, g : g + 1, :],
                                           ps_m[:, g : g + 1, :])
                            nc.vector.tensor_tensor(
                                net[:, g : g + 1, 1:17],
                                ps_p[:, g : g + 1, :],
                                sm[:, g : g + 1, :], OP.subtract,
                            )
                        m1 = pmpool.tile([16, GSET, 17], F32, tag="m1")
                        nc.tensor.matmul(
                            m1[:, ga:gb, :], tri[:, :], net[:, ga:gb, :],
                            start=True, stop=True,
                        )
                        fin = qpool.tile([16, GSET, 16], F32, tag="fin")
                        for gl in range(ga, gb):
                            nc.vector.scalar_tensor_tensor(
                                fin[:, gl, 0:16], net[:, gl, 0:16],
                                m1[:, gl, 16:17], m1[:, gl, 0:16],
                                OP.add, OP.subtract,
                            )
                        nc.sync.dma_start(
                            out_d.ap()[g0 + ga : g0 + gb, :].rearrange(
                                "g (K L) -> K g L", K=16
                            ),
                            fin[:, ga:gb, :],
                        )
                    elif last_half:
                        emit_matmuls(si, ga, gb, minus_first=True)
                        _finish((si, g0, ta, gb), tail=True)
                    else:
                        emit_matmuls(si, ga, gb, minus_first=MINUS_ALL)
                        if FIN_AFTER:
                            while pending:
                                _finish(pending.pop(0))
                        if FINISH_MODE == "set" and si < NSET - 1:
                            if hi == len(halves) - 1:
                                pending.append((si, g0, 0, GSET))
                        else:
                            pending.append((si, g0, ga, gb))
                for _ in range(NWARM_MID.get(si, 0)):
                    nc.tensor.matmul(
                        wps[:, :], wsrc[:, :], wsrc[:, :], start=True,
                        stop=True,
                    )
    nc.compile()
    return nc


_NC = None


def _get_nc():
    global _NC
    if _NC is None:
        _NC = build_nc()
    return _NC


def make_in_maps(pd0, pd1):
    pd0 = np.ascontiguousarray(np.asarray(pd0, dtype=np.float32))
    pd1 = np.ascontiguousarray(np.asarray(pd1, dtype=np.float32))
    tri = np.eye(16, k=1, dtype=np.float32)  # superdiag: m1[K,:] = net[K-1,:]
    bs = B // NCORES
    in_maps = []
    for c in range(NCORES):
        in_maps.append(
            {
                "pd0": np.ascontiguousarray(
                    pd0[bs * c : bs * (c + 1)].reshape(NG, N, 2)
                ),
                "pd1": np.ascontiguousarray(
                    pd1[bs * c : bs * (c + 1)].reshape(NG, N, 2)
                ),
                "tri": tri,
            }
        )
    return in_maps


def kernel(pd0, pd1, trace=False):
    nc = _get_nc()
    in_maps = make_in_maps(pd0, pd1)
    res = run_bass_kernel_spmd(nc, in_maps, list(range(NCORES)), trace=trace)
    bs = B // NCORES
    out = np.concatenate(
        [res.results[c]["out"].reshape(bs, C, TT) for c in range(NCORES)], axis=0
    )
    if trace:
        return out.astype(np.float32), res
    return out.astype(np.float32)
